# revision 31
# baseline (speedup 1.0000x reference)
"""Trainium2 Bass kernel for nn_DKOKernel (dense pairwise MLP + PSD head).

Math (per batch b, one NeuronCore per batch):
  hx[f,i] = sum_c wx[f,c] x[b,i,c];  hy[f,j] = sum_c wy[f,c] y[b,j,c]
  h1 = relu(bn1(hx_i + hy_j + b1))          (512)
  h2 = relu(bn2(W2 h1 + b2))                (256)
  h3 = relu(bn3(W3 h2 + b3))                (128)
  e  = W4 h3 + b4                           (64)
  out[b,i,j] = sum_k e[b,i,j,k] * (sum_l e[b,i,l,k])

BatchNorm affines folded into weights/biases on the host.

Head algebra (avoids materializing e):
  q_i  = sum_j h3_ij
  v_i  = M q_i + ny*wc          (M = W4^T W4, wc = W4^T b4)
  c_i  = wc.q_i + ny*|b4|^2
  out[i,j] = h3_ij . v_i + c_i

Device layout: features on partitions, (i-block, j) pairs on the free dim,
C=4 i-rows per chunk -> free 512.  Big matmuls in bf16 (~244ns/512-row MM
measured).  Output accumulated TRANSPOSED (outT[j,i]) in one held PSUM
bank via a ones-row matmul (c) + ap=1 matmuls (h3.v); host transposes.

Final design (~114us vs the 124.3us baseline; engines measured balanced
at ~2.9-3.0us/chunk each):
 - All big weights and x/y ship from the host in bf16 (half the DMA
   bytes, no device casts); setup matmuls run in bf16 (no fp32
   LOW_HIGH double passes).  Critical DMAs ride the HWDGE queues
   (sync/scalar); bulk weights the SWDGE gpsimd queue.
 - L1 (broadcast add + relu) is 16 [128,128] quarters per chunk,
   14 on DVE (~170ns, 2x mode) + 2 on ACT (~400ns) - the measured
   optimum; chunk 0 uses a wider ACT share while ACT is idle.
 - L3 accumulates into a [128,2,512] pair psum (2 banks, 2 bufs); the
   h3 relu-evict and q row-sum run once per chunk PAIR as [128,1024]
   ops, amortizing ACT's 352-cyc / DVE's 58-cyc per-op overheads.  The
   evict->reduce->head chain lags one chunk so the strict-FIFO engine
   queues never wait cross-engine.
 - KC_ORDER=3012 is load-bearing: L2 reads fc3 first while L1 writes
   fc0, avoiding SBUF port conflicts (0123 measured 19% slower).
 - Output accumulates transposed in one held psum bank (ones-row
   matmul adds c, ap=1 matmuls add h3.v), evicted in three slices
   (t=20, t=27, drain) to overlap the out DMA; the last pair's first
   half is evicted+reduced during the final chunk to shorten the tail.
"""

import numpy as np
import ml_dtypes
from contextlib import ExitStack

BF16NP = ml_dtypes.bfloat16

import concourse.bacc as bacc
import concourse.tile as tile
from concourse import mybir
from concourse.bass_utils import run_bass_kernel_spmd

F32 = mybir.dt.float32
BF16 = mybir.dt.bfloat16
AF = mybir.ActivationFunctionType
ALU = mybir.AluOpType
AX = mybir.AxisListType

EPS = 1e-5
B = 8
N = 128          # nx == ny
F = 128          # input feature dim
D1, D2, D3, D4 = 512, 256, 128, 64
C = 4            # i-rows per chunk -> free dim C*N = 512
NCH = N // C

import os
# engine per (fc, ii) L1 quarter, fc-major: V = DVE, A = ACT, G = GPSIMD
# emission during even chunks shares ACT's heavy slot (h3 pair evict):
# fewer A quarters there; odd-chunk emission gets more
L1_EVEN = os.environ.get('L1_EVEN', 'VVVVVVVVVVVVVVAA')
L1_ODD = os.environ.get('L1_ODD', 'VVVVVVVVVVVVVVAA')
# chunk-0 L1 runs while ACT is otherwise idle: wider ACT share
L1_ASSIGN_T0 = os.environ.get('L1_ASSIGN_T0', 'VVVVVVVVVVVAAAAA')
# engines for the two h2 psum evicts per chunk: V/A per slot
EV_ASSIGN = os.environ.get('EV_ASSIGN', 'AA')
# engine for the fused pair h3 evict (relu+bias [128,1024]): V/A
H3_ENGINE = os.environ.get('H3_ENGINE', 'A')
# L2 contraction order (which h1 fc group each accumulation step reads)
KC_ORDER = [int(c) for c in os.environ.get('KC_ORDER', '3012')]
# engine for the tiny head v/c evicts: A (ACT) or V (DVE)
HEADSMALL = os.environ.get('HEADSMALL', 'A')
# head group size in pairs: 1 = per-pair v/c, 2 = quad (one v/c per 16 i's)
QUAD = int(os.environ.get('QUAD', '1'))
DUMP = os.environ.get('DUMP', '')


def build_module():
    nc = bacc.Bacc()

    xyT = nc.declare_dram_parameter("xyT", [F, 2, N], BF16, isOutput=False)
    wxyT = nc.declare_dram_parameter("wxyT", [F, 2, D1], BF16, isOutput=False)
    w2T = nc.declare_dram_parameter("w2T", [128, 4, D2], BF16, isOutput=False)
    w3T = nc.declare_dram_parameter("w3T", [128, 2, D3], BF16, isOutput=False)
    Md = nc.declare_dram_parameter("M", [128, 128], F32, isOutput=False)
    Bd = nc.declare_dram_parameter("B", [128, 10], F32, isOutput=False)
    out_d = nc.declare_dram_parameter("out", [N, N], F32, isOutput=True)

    with tile.TileContext(nc) as tc:
        with ExitStack() as ctx:
            singles = ctx.enter_context(tc.tile_pool(name="singles", bufs=1))

            M_s = singles.tile([128, 128], F32)
            B_s = singles.tile([128, 10], F32)

            xyT_b = singles.tile([F, 2, N], BF16)
            wxyT_b = singles.tile([F, 2, D1], BF16)
            w2_b = singles.tile([128, 4, D2], BF16)
            w3_b = singles.tile([128, 2, D3], BF16)
            ones_b = singles.tile([1, 128], BF16)

            c1_s = B_s[:, 0:4]
            c2_s = B_s[:, 4:6]
            c3_s = B_s[:, 6:7]
            wc_s = B_s[:, 7:8]
            nywc_s = B_s[:, 8:9]
            c0_s = B_s[0:1, 9:10]

            hx_s = singles.tile([128, 4, N], F32)    # [f, fc, i] scalars
            hy_b = singles.tile([128, 4, N], BF16)   # [f, fc, j] (+c1)
            q_all = singles.tile([128, N], F32)      # per-i row-sums of h3
            outT_s = singles.tile([N, N], F32)

            # ---- DMAs (bf16 weights prepacked on host): critical path on
            # the HWDGE queues (sync/scalar), bulk on the SWDGE gpsimd ----
            nc.sync.dma_start(out=wxyT_b[:, 0, :], in_=wxyT[:, 0, :])
            nc.scalar.dma_start(out=xyT_b, in_=xyT[:, :, :])
            nc.sync.dma_start(out=wxyT_b[:, 1, :], in_=wxyT[:, 1, :])
            nc.scalar.dma_start(out=B_s, in_=Bd[:, :])
            nc.gpsimd.dma_start(out=w2_b[:, 3, :], in_=w2T[:, 3, :])
            nc.sync.dma_start(out=w2_b[:, 0:2, :], in_=w2T[:, 0:2, :])
            nc.gpsimd.dma_start(out=w2_b[:, 2, :], in_=w2T[:, 2, :])
            nc.scalar.dma_start(out=w3_b, in_=w3T[:, :, :])
            nc.gpsimd.dma_start(out=M_s, in_=Md[:, :])
            nc.vector.memset(ones_b, 1.0)

            # ---- setup: hx (DVE evicts), hy+c1 (ACT evicts), bf16 mms ----
            with tc.tile_pool(name="psum_setup", bufs=2, space="PSUM") as pp:
                for fc in range(4):
                    ph = pp.tile([128, N], F32, tag="ps", name="ph")
                    nc.tensor.matmul(
                        ph, lhsT=wxyT_b[:, 0, fc * 128:(fc + 1) * 128],
                        rhs=xyT_b[:, 0, :], start=True, stop=True)
                    nc.vector.tensor_copy(out=hx_s[:, fc, :], in_=ph)
                    py_ = pp.tile([128, N], F32, tag="ps2", name="py_")
                    nc.tensor.matmul(
                        py_, lhsT=wxyT_b[:, 1, fc * 128:(fc + 1) * 128],
                        rhs=xyT_b[:, 1, :], start=True, stop=True)
                    if fc % 2 == 0:
                        # split the hy evicts across DVE/ACT: shortens the
                        # serial ACT chain gating the first L1 quarters
                        nc.vector.tensor_scalar(
                            out=hy_b[:, fc, :], in0=py_,
                            scalar1=c1_s[:, fc:fc + 1], scalar2=None,
                            op0=ALU.add)
                    else:
                        nc.scalar.activation(hy_b[:, fc, :], py_,
                                             AF.Identity,
                                             bias=c1_s[:, fc:fc + 1])

            work = ctx.enter_context(tc.tile_pool(name="work", bufs=3))
            h3pool = ctx.enter_context(tc.tile_pool(name="h3p", bufs=3))
            psum2 = ctx.enter_context(tc.tile_pool(name="psum2", bufs=2,
                                                   space="PSUM"))
            psum3 = ctx.enter_context(tc.tile_pool(name="psum3", bufs=2,
                                                   space="PSUM"))
            psumh = ctx.enter_context(tc.tile_pool(name="psumh", bufs=1,
                                                   space="PSUM"))
            psumo = ctx.enter_context(tc.tile_pool(name="psumo", bufs=1,
                                                   space="PSUM"))
            poT = psumo.tile([N, N], F32)    # held outT[j, i] accumulator

            def emit_L1(t):
                h1 = work.tile([128, 4, C * N], BF16, tag="h1", name="h1")
                for fc in range(4):
                    for ii in range(C):
                        sl = slice(ii * N, (ii + 1) * N)
                        xc = hx_s[:, fc, C * t + ii:C * t + ii + 1]
                        if t == 0:
                            amap = L1_ASSIGN_T0
                        else:
                            amap = L1_EVEN if (t - 1) % 2 == 0 else L1_ODD
                        mode = amap[fc * C + ii]
                        if mode == "V":
                            nc.vector.tensor_scalar(
                                out=h1[:, fc, sl], in0=hy_b[:, fc, :],
                                scalar1=xc, scalar2=0.0,
                                op0=ALU.add, op1=ALU.max)
                        else:
                            nc.scalar.activation(
                                h1[:, fc, sl], hy_b[:, fc, :],
                                AF.Relu, bias=xc)
                return h1

            def emit_head_vc(tq, W):
                # v/c for a group of W i's starting at i = W*... (quad: W=16)
                q_sl = q_all[:, W * tq:W * tq + W]
                ps_vc = psumh.tile([128, 2 * W], F32, tag="hv", name="ps_vc")
                ps_v = ps_vc[:, 0:W]
                ps_c = ps_vc[0:1, W:2 * W]
                nc.tensor.matmul(ps_v, lhsT=M_s, rhs=q_sl,
                                 start=True, stop=True)
                v_sb = work.tile([128, 16], BF16, tag="v", name="v_sb")
                nc.tensor.matmul(ps_c, lhsT=wc_s, rhs=q_sl,
                                 start=True, stop=True)
                c_sb = work.tile([1, 16], BF16, tag="c", name="c_sb")
                if HEADSMALL == 'V':
                    nc.vector.tensor_scalar(
                        out=v_sb[:, 0:W], in0=ps_v, scalar1=nywc_s,
                        scalar2=None, op0=ALU.add)
                    nc.vector.tensor_scalar(
                        out=c_sb[:, 0:W], in0=ps_c, scalar1=c0_s,
                        scalar2=None, op0=ALU.add)
                else:
                    nc.scalar.activation(v_sb[:, 0:W], ps_v, AF.Identity,
                                         bias=nywc_s)
                    nc.scalar.activation(c_sb[:, 0:W], ps_c, AF.Identity,
                                         bias=c0_s)
                return (v_sb, c_sb)

            def emit_head_quad(quad):
                # one ones-row matmul for the whole group, then per-pair
                # ap=1 matmuls; quad = list of (tp, h3_pair)
                W = 8 * len(quad)
                base = 8 * quad[0][0]
                v_sb, c_sb = emit_head_vc(base // W, W)
                po_q = poT[:, base:base + W]
                nc.tensor.matmul(po_q, lhsT=ones_b, rhs=c_sb[:, 0:W],
                                 start=True, stop=False)
                for o, (tp, h3_pair) in enumerate(quad):
                    for k in range(2):
                        for a in range(C):
                            col = o * 8 + k * C + a
                            nc.tensor.matmul(
                                po_q[:, col:col + 1],
                                lhsT=h3_pair[:, k, a * N:(a + 1) * N],
                                rhs=v_sb[:, col:col + 1],
                                start=False,
                                stop=(o == len(quad) - 1 and k == 1
                                      and a == C - 1))

            def emit_evict_q(tp, p3_t):
                # fused pair evict: relu+bias over [128, 1024], then q sums
                h3_pair = h3pool.tile([128, 2, C * N], BF16, tag="h3",
                                      name="h3_pair")
                if H3_ENGINE == "A":
                    nc.scalar.activation(h3_pair, p3_t, AF.Relu, bias=c3_s)
                else:
                    nc.vector.tensor_scalar(
                        out=h3_pair, in0=p3_t, scalar1=c3_s,
                        scalar2=0.0, op0=ALU.add, op1=ALU.max)
                nc.vector.tensor_reduce(
                    out=q_all[:, 2 * C * tp:2 * C * (tp + 1)],
                    in_=h3_pair.rearrange("p k (a b) -> p (k a) b", b=N),
                    axis=AX.X, op=ALU.add)
                return h3_pair

            def emit_q_half1(tp, h3_pair):
                nc.vector.tensor_reduce(
                    out=q_all[:, 2 * C * tp + C:2 * C * (tp + 1)],
                    in_=h3_pair[:, 1, :].rearrange("p (a b) -> p a b", b=N),
                    axis=AX.X, op=ALU.add)

            evq = None       # (tp, p3_tile) awaiting evict+reduce
            pend_pairs = []  # (tp, h3_pair) awaiting head emission
            p3_t = None
            h1_cur = emit_L1(0)
            for t in range(NCH):
                # lag-1: evict+reduce the pair finished in chunk t-1
                if evq is not None and t % 2 == 0:
                    tp_e, p3_e = evq
                    pend_pairs.append((tp_e, emit_evict_q(tp_e, p3_e)))
                    evq = None

                h1_next = emit_L1(t + 1) if t + 1 < NCH else None

                # ---- L2 ----
                h2 = work.tile([128, 2, C * N], BF16, tag="h2", name="h2")
                for mc in range(2):
                    p2 = psum2.tile([128, C * N], F32, tag="p2", name="p2")
                    for i_kc, kc in enumerate(KC_ORDER):
                        nc.tensor.matmul(
                            p2, lhsT=w2_b[:, kc, mc * 128:(mc + 1) * 128],
                            rhs=h1_cur[:, kc, :],
                            start=(i_kc == 0), stop=(i_kc == 3))
                    if EV_ASSIGN[mc] == "A":
                        nc.scalar.activation(h2[:, mc, :], p2, AF.Relu,
                                             bias=c2_s[:, mc:mc + 1])
                    else:
                        nc.vector.tensor_scalar(
                            out=h2[:, mc, :], in0=p2,
                            scalar1=c2_s[:, mc:mc + 1], scalar2=0.0,
                            op0=ALU.add, op1=ALU.max)

                # ---- L3 into a pair-double psum bank ----
                par = t % 2
                if par == 0:
                    p3_t = psum3.tile([128, 2, C * N], F32, tag="p3",
                                      name="p3_t")
                for kc in range(2):
                    nc.tensor.matmul(
                        p3_t[:, par, :], lhsT=w3_b[:, kc, :],
                        rhs=h2[:, kc, :], start=(kc == 0), stop=(kc == 1))

                if DUMP:
                    parts = DUMP.split(':')
                    dt_, idx, td = parts[0], int(parts[1]), int(parts[2])
                    if t == td:
                        src = {'h1': h1_cur[:, idx, 0:N],
                               'h2': h2[:, idx, 0:N]}[dt_]
                        nc.vector.tensor_copy(out=outT_s[:, 0:N], in_=src)

                # head for a completed group of evicted+reduced pairs
                if len(pend_pairs) >= QUAD and t % 2 == 1:
                    emit_head_quad(pend_pairs[:QUAD])
                    pend_pairs = pend_pairs[QUAD:]
                if not DUMP and t == 27:
                    # third quarter of poT complete (heads done by t=25)
                    nc.vector.tensor_copy(out=outT_s[:, N // 2:3 * N // 4],
                                          in_=poT[:, N // 2:3 * N // 4])
                    nc.sync.dma_start(out=out_d[:, N // 2:3 * N // 4],
                                      in_=outT_s[:, N // 2:3 * N // 4])
                if not DUMP and t == NCH // 2 + 4:
                    # first half of poT is complete (head lags two pairs)
                    nc.vector.tensor_copy(out=outT_s[:, 0:N // 2],
                                          in_=poT[:, 0:N // 2])
                    nc.sync.dma_start(out=out_d[:, 0:N // 2],
                                      in_=outT_s[:, 0:N // 2])
                if par == 1 and not DUMP:
                    evq = (t // 2, p3_t)
                if t == NCH - 1 and not DUMP:
                    # last pair: evict+reduce its first half now so the
                    # post-loop drain chain only handles half 1
                    tp_l, p3_l = evq
                    h3_last = h3pool.tile([128, 2, C * N], BF16, tag="h3",
                                          name="h3_last")
                    if H3_ENGINE == "A":
                        nc.scalar.activation(h3_last[:, 0, :], p3_l[:, 0, :],
                                             AF.Relu, bias=c3_s)
                    else:
                        nc.vector.tensor_scalar(
                            out=h3_last[:, 0, :], in0=p3_l[:, 0, :],
                            scalar1=c3_s, scalar2=0.0,
                            op0=ALU.add, op1=ALU.max)
                    nc.vector.tensor_reduce(
                        out=q_all[:, 2 * C * tp_l:2 * C * tp_l + C],
                        in_=h3_last[:, 0, :].rearrange("p (a b) -> p a b",
                                                       b=N),
                        axis=AX.X, op=ALU.add)
                    evq = ('half', tp_l, p3_l, h3_last)
                h1_cur = h1_next

            # drain: finish the last pair's evict, then the final quad
            if evq is not None:
                if evq[0] == 'half':
                    _, tp_e, p3_e, h3p_h = evq
                    if H3_ENGINE == "A":
                        nc.scalar.activation(h3p_h[:, 1, :], p3_e[:, 1, :],
                                             AF.Relu, bias=c3_s)
                    else:
                        nc.vector.tensor_scalar(
                            out=h3p_h[:, 1, :], in0=p3_e[:, 1, :],
                            scalar1=c3_s, scalar2=0.0,
                            op0=ALU.add, op1=ALU.max)
                    nc.vector.tensor_reduce(
                        out=q_all[:, 2 * C * tp_e + C:2 * C * (tp_e + 1)],
                        in_=h3p_h[:, 1, :].rearrange("p (a b) -> p a b",
                                                     b=N),
                        axis=AX.X, op=ALU.add)
                else:
                    tp_e, p3_e = evq
                    h3p_h = emit_evict_q(tp_e, p3_e)
                pend_pairs.append((tp_e, h3p_h))
                evq = None
            while pend_pairs:
                emit_head_quad(pend_pairs[:QUAD])
                pend_pairs = pend_pairs[QUAD:]
            nc.vector.tensor_copy(out=outT_s[:, 3 * N // 4:],
                                   in_=poT[:, 3 * N // 4:])
            nc.sync.dma_start(out=out_d[:, 3 * N // 4:],
                              in_=outT_s[:, 3 * N // 4:])
    nc.finalize()
    return nc


_NC_CACHE = None


def _get_nc():
    global _NC_CACHE
    if _NC_CACHE is None:
        _NC_CACHE = build_module()
    return _NC_CACHE


def host_prep(inputs):
    """Fold the BatchNorm affines into weights/biases; pre-transpose
    everything into the device layouts. Returns the per-core input maps."""
    f32 = np.float32
    x = np.asarray(inputs["x"], f32)
    y = np.asarray(inputs["y"], f32)
    w1, b1 = np.asarray(inputs["w1"], f32), np.asarray(inputs["b1"], f32)
    w2, b2 = np.asarray(inputs["w2"], f32), np.asarray(inputs["b2"], f32)
    w3, b3 = np.asarray(inputs["w3"], f32), np.asarray(inputs["b3"], f32)
    w4, b4 = np.asarray(inputs["w4"], f32), np.asarray(inputs["b4"], f32)

    k1 = inputs["g1"] / np.sqrt(inputs["v1"] + EPS)
    c1 = k1 * (b1 - inputs["m1"]) + inputs["be1"]
    k2 = inputs["g2"] / np.sqrt(inputs["v2"] + EPS)
    c2 = k2 * (b2 - inputs["m2"]) + inputs["be2"]
    k3 = inputs["g3"] / np.sqrt(inputs["v3"] + EPS)
    c3 = k3 * (b3 - inputs["m3"]) + inputs["be3"]

    wx = w1[:, :F] * k1[:, None]          # (512, 128)
    wy = w1[:, F:] * k1[:, None]
    w2f = w2 * k2[:, None]                # (256, 512)
    w3f = w3 * k3[:, None]                # (128, 256)

    Bm = np.zeros((128, 10), f32)
    Bm[:, 0:4] = c1.reshape(4, 128).T
    Bm[:, 4:6] = c2.reshape(2, 128).T
    Bm[:, 6] = c3
    Bm[:, 7] = w4.T @ b4
    Bm[:, 8] = N * (w4.T @ b4)
    Bm[0, 9] = N * float(b4 @ b4)

    wxy = np.stack([wx.T, wy.T], axis=1)  # (128, 2, 512)

    shared = {
        "wxyT": np.ascontiguousarray(wxy).astype(BF16NP),
        "w2T": np.ascontiguousarray(
            w2f.T.reshape(4, 128, D2).transpose(1, 0, 2)).astype(BF16NP),
        "w3T": np.ascontiguousarray(
            w3f.T.reshape(2, 128, D3).transpose(1, 0, 2)).astype(BF16NP),
        "M": np.ascontiguousarray(w4.T @ w4, f32),                 # (128, 128)
        "B": Bm,
    }
    in_maps = []
    for b in range(B):
        m = dict(shared)
        m["xyT"] = np.ascontiguousarray(
            np.stack([x[b].T, y[b].T], axis=1)).astype(BF16NP)  # (128, 2, 128)
        in_maps.append(m)
    return in_maps


def gather(res, inputs):
    """Device returns outT[j,i]; transpose back per batch."""
    outs = [res.results[b]["out"].T for b in range(B)]
    return np.ascontiguousarray(np.stack(outs, axis=0), np.float32)


def kernel(**inputs):
    nc = _get_nc()
    in_maps = host_prep(inputs)
    res = run_bass_kernel_spmd(nc, in_maps, list(range(B)))
    return gather(res, inputs)
